# revision 47
# baseline (speedup 1.0000x reference)
"""MoE (BruteForceMoELinear) Trainium2 kernel.

Strategy: expert-parallel across 8 NeuronCores. The host (inside
`kernel()`) dispatches token rows by `gate_idx` (stable sort), pads each
expert's token batch to a common capacity C, and hands core e:

  xt  : (128, 4, C)    = x_e^T   laid out [d_inner, d_outer, token]
  w1t : (128, 4, 2048) = W1_e^T  laid out [d_inner, d_outer, f]
  w2t : (128, 16, 512) = W2_e^T  laid out [f_inner, f_outer, d_out]
  sc  : (128, C)       = per-token gate score, replicated over partitions

Each core computes  y_e^T = (W2_e @ relu(W1_e @ x_e^T)) * score  with
float32r matmuls (full-rate fp32 PE path), ReLU fused into the PSUM
eviction on the scalar engine and the gate-score multiply fused into the
second GEMM's PSUM eviction on the vector engine.  The host scatters the
per-expert outputs back to token order and sums the top-k (=2) slots.
"""

import numpy as np

NUM_EXPERT = 8
N_CORES = 8
P = 128

_CACHE = {}


def _build(TN, NCH, KO, FO, repeat=1):
    """Compile the per-core program for capacity C = TN*NCH tokens.

    KO = d_model/128, FO = d_ff/128.  `repeat` re-emits the compute body
    (used only for timing calibration in the dev harness).
    """
    key = (TN, NCH, KO, FO, repeat)
    if key in _CACHE:
        return _CACHE[key]

    import concourse.mybir as mybir
    import concourse.tile as tile
    from concourse import bacc

    f32 = mybir.dt.float32
    f32r = mybir.dt.float32r
    C = TN * NCH
    D_MODEL = KO * P
    D_FF = FO * P

    nc = bacc.Bacc("TRN2", target_bir_lowering=False, debug=False,
                   num_devices=N_CORES)

    xt = nc.dram_tensor("xt", (P, KO, C), f32r, kind="ExternalInput")
    w1t = nc.dram_tensor("w1t", (P, KO, D_FF), f32r, kind="ExternalInput")
    w2t = nc.dram_tensor("w2t", (P, FO, D_MODEL), f32r, kind="ExternalInput")
    sc = nc.dram_tensor("sc", (P, C), f32, kind="ExternalInput")
    yt = nc.dram_tensor("yt", (P, KO, C), f32, kind="ExternalOutput")

    # Holding every chunk's h in SBUF only fits for NCH <= 2; for heavily
    # skewed expert distributions (NCH >= 3) process chunk-major with a
    # rotating 2-buffer h pool instead.
    NHB = NCH if NCH <= 2 else 2
    NXB = NCH if NCH <= 2 else 3
    with tile.TileContext(nc) as tc:
        with tc.tile_pool(name="wpool", bufs=1) as wpool, \
             tc.tile_pool(name="xpool", bufs=NXB) as xpool, \
             tc.tile_pool(name="hpool", bufs=NHB) as hpool, \
             tc.tile_pool(name="ypool", bufs=4) as ypool, \
             tc.tile_pool(name="cpool", bufs=1) as cpool, \
             tc.tile_pool(name="ps1", bufs=4, space="PSUM") as ps1, \
             tc.tile_pool(name="ps2", bufs=4, space="PSUM") as ps2:

            bias0 = cpool.tile([P, 1], f32)
            nc.any.memset(bias0[:], 0.0)

            # PE warm-up: dummy matmuls on memset data keep the PE busy
            # through the DMA-priming window so the HAM clock gate is at
            # full rate when the first real matmul issues.
            warm = cpool.tile([P, 64], f32)
            nc.any.memset(warm[:], 0.5)
            wps = ps1.tile([P, 64], f32, name="warm", tag="p1")
            for _i in range(20):
                nc.tensor.matmul(wps[:64, :], warm[:], warm[:],
                                 start=True, stop=True)

            # DMAs execute in emission order on the DMA stream, which is
            # the pacing resource at kernel start.  Emit strictly in
            # consumption order: x(ch0) -> W1 -> x(ch1..) -> W2/sc.
            w1sb = wpool.tile([P, KO, D_FF], f32r)
            w2sb = wpool.tile([P, FO, D_MODEL], f32r)
            scsb = cpool.tile([P, C], f32)
            if NCH <= 2:
                xsbs = [xpool.tile([P, KO, TN], f32r, tag=f"x{ch}",
                                   name=f"xsb{ch}") for ch in range(NCH)]
            else:
                xsbs = None  # allocated per chunk in the fallback loop

            # DMA emission order == consumption order: x/W1 for the first
            # f-block pairwise (fine-grained so the first fo-group starts
            # after ~3 small DMAs), later chunks' x, the rest of W1, then
            # W2 d-blocks and the gate scores.
            FB = 512
            NFB = D_FF // FB
            FPB = FB // P  # fo-groups per W1 f-block
            if NCH <= 2:
                for kh in range(KO // 2):
                    k0, k1 = 2 * kh, 2 * kh + 2
                    nc.sync.dma_start(w1sb[:, k0:k1, 0:FB],
                                      w1t.ap()[:, k0:k1, 0:FB])
                    nc.sync.dma_start(xsbs[0][:, k0:k1, :],
                                      xt.ap()[:, k0:k1, 0:TN])
                for ch in range(1, NCH):
                    nc.sync.dma_start(xsbs[ch][:],
                                      xt.ap()[:, :, ch * TN:(ch + 1) * TN])
            else:
                nc.sync.dma_start(w1sb[:, :, 0:FB], w1t.ap()[:, :, 0:FB])
            HB = FB // 4
            for hb in range(4, 4 * NFB):
                nc.sync.dma_start(
                    w1sb[:, :, hb * HB:(hb + 1) * HB],
                    w1t.ap()[:, :, hb * HB:(hb + 1) * HB])
            FH = FO // 2
            nc.sync.dma_start(w2sb[:, 0:FH, 0:P], w2t.ap()[:, 0:FH, 0:P])
            nc.sync.dma_start(w2sb[:, FH:, 0:P], w2t.ap()[:, FH:, 0:P])
            nc.sync.dma_start(scsb[:], sc.ap())
            for db in range(1, KO):
                nc.sync.dma_start(w2sb[:, 0:FH, db * P:(db + 1) * P],
                                  w2t.ap()[:, 0:FH, db * P:(db + 1) * P])
                nc.sync.dma_start(w2sb[:, FH:, db * P:(db + 1) * P],
                                  w2t.ap()[:, FH:, db * P:(db + 1) * P])

            def gemm1(hsb, xsb, fo):
                p1 = ps1.tile([P, TN], f32, name="p1", tag="p1")
                for ko in range(KO):
                    nc.tensor.matmul(
                        p1[:],
                        w1sb[:, ko, fo * P:(fo + 1) * P],
                        xsb[:, ko, :],
                        start=(ko == 0), stop=(ko == KO - 1))
                nc.scalar.activation(
                    hsb[:, fo, :], p1[:],
                    mybir.ActivationFunctionType.Relu, bias=bias0[:])

            def gemm2(hsb, do, tsl):
                p2 = ps2.tile([P, TN], f32, name="p2", tag="p2")
                for fo in range(FO):
                    nc.tensor.matmul(
                        p2[:],
                        w2sb[:, fo, do * P:(do + 1) * P],
                        hsb[:, fo, :],
                        start=(fo == 0), stop=(fo == FO - 1))
                ysb = ypool.tile([P, TN], f32, tag="y", name="ysb")
                nc.vector.tensor_mul(ysb[:], p2[:], scsb[:, tsl])
                nc.sync.dma_start(yt.ap()[:, do, tsl], ysb[:])

            for _ in range(repeat):
                if NCH <= 2:
                    hsbs = [hpool.tile([P, FO, TN], f32r, tag=f"h{ch}",
                                       name=f"hsb{ch}") for ch in range(NCH)]
                    # phase 1: h = relu(W1 @ x^T); f-block-major so every
                    # W1 block feeds all chunks' matmuls before the next
                    # block is needed (keeps PE ahead of the DMA stream).
                    for fb in range(NFB):
                        for ch in range(NCH):
                            for fo in range(fb * FPB, (fb + 1) * FPB):
                                gemm1(hsbs[ch], xsbs[ch], fo)
                    # phase 2: y^T = (W2 @ h) * score; d-block-major,
                    # streamed out per (db, chunk).
                    for do in range(KO):
                        for ch in range(NCH):
                            gemm2(hsbs[ch], do,
                                  slice(ch * TN, (ch + 1) * TN))
                else:
                    # chunk-major fallback (bounded SBUF for large NCH)
                    for ch in range(NCH):
                        xsb = xpool.tile([P, KO, TN], f32r, tag="x",
                                         name="xsb")
                        nc.sync.dma_start(
                            xsb[:], xt.ap()[:, :, ch * TN:(ch + 1) * TN])
                        hsb = hpool.tile([P, FO, TN], f32r, tag="h",
                                         name="hsb")
                        for fo in range(FO):
                            gemm1(hsb, xsb, fo)
                        for do in range(KO):
                            gemm2(hsb, do, slice(ch * TN, (ch + 1) * TN))

    nc.compile()
    _CACHE[key] = nc
    return nc


def _capacity(max_count):
    """Chunking: NCH chunks of TN tokens; TN in [256, 512] keeps the
    float32r matmul at full rate and within one PSUM bank."""
    maxc = max(int(max_count), 1)
    nch = -(-maxc // 512)
    tn = -(-maxc // (nch * 8)) * 8
    tn = max(tn, 256)
    return tn, nch


_last = {}


def kernel(inp, gate_idx, gate_score, w_htoh4, w_h4toh):
    inp = np.ascontiguousarray(np.asarray(inp, dtype=np.float32))
    gate_idx = np.asarray(gate_idx)
    gate_score = np.asarray(gate_score, dtype=np.float32)
    w_htoh4 = np.asarray(w_htoh4, dtype=np.float32)
    w_h4toh = np.asarray(w_h4toh, dtype=np.float32)

    B, d_model = inp.shape
    n_expert, d_ff, _ = w_htoh4.shape
    assert n_expert == NUM_EXPERT
    KO = d_model // P
    FO = d_ff // P

    gi = gate_idx.astype(np.int64)
    order = np.argsort(gi, kind="stable")
    counts = np.bincount(gi, minlength=NUM_EXPERT)
    idx_split = np.split(order, np.cumsum(counts)[:-1])

    TN, NCH = _capacity(counts.max())
    C = TN * NCH

    # flat per-row gate scores: row 2n+k of inp gets gate_score[n, 0, k]
    scores_flat = gate_score.reshape(-1)

    nc = _build(TN, NCH, KO, FO)

    in_maps = []
    for e in range(NUM_EXPERT):
        idx = idx_split[e]
        cnt = len(idx)
        xT = np.zeros((d_model, C), dtype=np.float32)
        if cnt:
            xT[:, :cnt] = inp[idx].T
        xt_h = np.ascontiguousarray(
            xT.reshape(KO, P, C).transpose(1, 0, 2))
        w1_h = np.ascontiguousarray(
            w_htoh4[e].T.reshape(KO, P, d_ff).transpose(1, 0, 2))
        w2_h = np.ascontiguousarray(
            w_h4toh[e].T.reshape(FO, P, d_model).transpose(1, 0, 2))
        sc_vec = np.zeros((C,), dtype=np.float32)
        if cnt:
            sc_vec[:cnt] = scores_flat[idx]
        sc_h = np.ascontiguousarray(np.broadcast_to(sc_vec, (P, C)))
        in_maps.append({"xt": xt_h, "w1t": w1_h, "w2t": w2_h, "sc": sc_h})

    from concourse import bass_utils
    res = bass_utils.run_bass_kernel_spmd(nc, in_maps,
                                          core_ids=list(range(N_CORES)))

    _last.update(nc=nc, in_maps=in_maps, res=res, TN=TN, NCH=NCH,
                 KO=KO, FO=FO)

    y_full = np.empty((B, d_model), dtype=np.float32)
    for e in range(NUM_EXPERT):
        idx = idx_split[e]
        if len(idx) == 0:
            continue
        yt_h = res.results[e]["yt"]  # (P, KO, C)
        yT = yt_h.transpose(1, 0, 2).reshape(d_model, C)
        y_full[idx] = yT[:, :len(idx)].T

    out = y_full[0::2] + y_full[1::2]
    return np.ascontiguousarray(out, dtype=np.float32)
